# revision 38
# baseline (speedup 1.0000x reference)
"""Trainium2 Bass kernel for nn_ATTConv (per-node attention over 3 neighbor
aggregates + center, per-type Linear(2D->1) scorer, LeakyReLU, softmax,
weighted sum).

Sharding: data-parallel over the node axis B across 8 cores; per-type
attention weights replicated. Inputs/outputs cross HBM as fp16 (host casts,
~8e-4 absmax-relative error); on-chip accumulation is fp32.

Host pads each core's shard to BSP=12544 rows (98 full 128-row subtiles;
pad rows are computed and dropped).

Layout: nodes on SBUF partitions (128 per subtile), D on the free axis.
Engine split per 512-node chunk:
  - all 5 score dots on PE: transpose the 4 candidate tiles -> PSUM
    (explicit start/stop flags; default-None is an order of magnitude
    slower on HW), copy to SBUF fp16 (ScalarE + VectorE), 4 accumulating
    matmuls with selector-column weights (d on partitions) produce the
    [5, N] score rows in one PSUM tile, small transposes bring the
    scores back node-major
  - softmax: LeakyReLU as one exact fused VectorE op max(slope*y, y),
    ScalarE Exp, VectorE reduce + reciprocal
  - weighted aggregation: PE matmuls with diagonal lhsT = diag(attn_c)
    fp16 built on VectorE (tensor_scalar on an identity tile),
    accumulated in PSUM fp32 (psum[n,d] += attn_c[n] * E_c[n,d]),
    8 subtiles per PSUM tile
  - PSUM -> SBUF fp16 via VectorE copy, then DMA out
"""

import numpy as np

T = 3
B = 100000
D = 128
NCORES = 8
BS = B // NCORES  # 12500 rows per core per type
BSP = 12544  # padded to 98 full 128-row subtiles (pad rows are dropped)
GROUP = 1024  # rows per processing group (8 subtiles of 128)

NEG_SLOPE = 0.01

_cache = {}


def _groups(total, group):
    out = []
    r0 = 0
    while total - r0 >= group:
        out.append((r0, group))
        r0 += group
    rem = total - r0
    if rem >= 128:
        full = (rem // 128) * 128
        out.append((r0, full))
        r0 += full
        rem -= full
    if rem:
        out.append((r0, rem))
    return out


def build_nc(bs=BSP, group=GROUP, repeat=1, ep_bufs=10, sp_bufs=12, dp_bufs=24, pp_bufs=1, sct_bufs=1, et_bufs=4, pt_bufs=4, sc_bufs=1, op_bufs=4, ssp_bufs=4, oc_act=0, agg_w=8, mode="full"):
    import concourse.bacc as bacc
    import concourse.tile as tile
    from concourse import mybir
    import concourse.bass as bass
    from concourse.masks import make_identity

    f32 = mybir.dt.float32
    f16 = mybir.dt.float16
    nc = bacc.Bacc("TRN2", target_bir_lowering=False, debug=False)

    hc = nc.dram_tensor("h_center", [T, bs, D], f16, kind="ExternalInput")
    hn = nc.dram_tensor("h_neigh", [T, T, bs, D], f16, kind="ExternalInput")
    aw = nc.dram_tensor("att_w", [T, 2 * D], f32, kind="ExternalInput")
    ab = nc.dram_tensor("att_b", [T], f32, kind="ExternalInput")
    out = nc.dram_tensor("out", [T, bs, D], f16, kind="ExternalOutput")

    NC = T + 1  # candidates: 3 neighbor types + center

    with tile.TileContext(nc) as tc:
        with (
            tc.tile_pool(name="const", bufs=1) as const,
            tc.tile_pool(name="ep", bufs=ep_bufs) as ep,
            tc.tile_pool(name="tp", bufs=2) as tp,
            tc.tile_pool(name="sp", bufs=sp_bufs) as sp,
            tc.tile_pool(name="dp", bufs=dp_bufs) as dp,
            tc.tile_pool(name="op", bufs=op_bufs) as op,
            tc.tile_pool(name="etp", bufs=et_bufs) as etp,
            tc.tile_pool(name="ssp", bufs=ssp_bufs) as ssp,
            tc.tile_pool(name="ptps", bufs=pt_bufs, space="PSUM") as ptps,
            tc.tile_pool(name="scps", bufs=sc_bufs, space="PSUM") as scps,
            tc.tile_pool(name="sctps", bufs=sct_bufs, space="PSUM") as sctps,
            tc.tile_pool(name="pp", bufs=pp_bufs, space="PSUM") as pp,
        ):
            # --- constants -------------------------------------------------
            ident = const.tile([128, 128], f16)
            make_identity(nc, ident[:, :])
            identf = const.tile([8, 8], f32)
            make_identity(nc, identf[:, :])

            aw_ap = aw.ap()
            # wte_sel[d, t, k, m] fp16 selector weights (d on partitions):
            # k in 0..3: column m == k holds w_e(t) (candidate scores into
            # PSUM row k); k == 4: column 4 holds w_h(t) (center w_h score
            # into row 4). Other columns are zero so the 5 accumulating
            # matmuls write disjoint rows of one [5, N] PSUM tile (matmul
            # PSUM outputs must start at partition 0).
            NS = NC + 1  # 5 score rows
            # k in 0..2: column k holds w_e(t); k == 3 (the center matmul)
            # fills BOTH column 3 (w_e) and column 4 (w_h) so one pass over
            # the transposed center tile yields rows 3 and 4 together.
            wte_sel = const.tile([128, T, NC, NS], f16)
            nc.gpsimd.memset(wte_sel[:, :, :, :], 0.0)
            for t in range(T):
                for k in range(NC):
                    nc.gpsimd.dma_start(
                        out=wte_sel[:, t, k, k : k + 1],
                        in_=bass.AP(
                            tensor=aw_ap.tensor,
                            offset=t * 2 * D + D,
                            ap=[[1, 128], [1, 1]],
                        ),
                    )
                nc.gpsimd.dma_start(
                    out=wte_sel[:, t, NC - 1, NC : NC + 1],
                    in_=bass.AP(
                        tensor=aw_ap.tensor,
                        offset=t * 2 * D,
                        ap=[[1, 128], [1, 1]],
                    ),
                )
            bias = const.tile([128, T], f32)
            nc.gpsimd.dma_start(
                out=bias[:, :],
                in_=bass.AP(tensor=ab.ap().tensor, offset=0, ap=[[0, 128], [1, T]]),
            )

            # --- main loop -------------------------------------------------
            oc_i = [0]
            for _rep in range(repeat):
              for t in range(T):
                for r0, nr in _groups(bs, group):
                    S = (nr + 127) // 128
                    pfull = nr % 128 == 0
                    assert pfull, "bs must be a multiple of 128"
                    pmax = 128

                    # load candidate streams (fp16, S consecutive rows/part)
                    etiles = []
                    for c in range(NC):
                        E = ep.tile([128, S, D], f16, tag=f"E{c}")
                        src = (
                            hn.ap()[t, c, r0 : r0 + nr, :]
                            if c < T
                            else hc.ap()[t, r0 : r0 + nr, :]
                        )
                        nc.sync.dma_start(
                            out=E[:, :, :],
                            in_=src.rearrange("(p s) d -> p s d", p=128),
                        )
                        etiles.append(E)

                    # sc_e[:, :, 0:3]: neighbor scores (PE path)
                    # sc_e[:, :, 3]: center w_e score; sc_e[:, :, 4]: w_h
                    sc_e = sp.tile([128, S, NC + 1], f32, tag="sc_e")

                    # --- all 5 score dots on PE (per 512-node chunk) ------
                    if mode == "noscore":
                        nc.vector.memset(sc_e[0:pmax], 0.5)
                    elif mode == "dma":
                        pass
                    else:
                        for s4 in range(0, S, 4):
                            sn = min(4, S - s4)
                            sc = scps.tile([8, 4 * D], f32, tag="sc")
                            et = etp.tile([128, NC, 4, D], f16, tag="et")
                            for g in range(2):
                                pt = ptps.tile([128, 2, 4, D], f16, tag="pt")
                                for ci in range(2):
                                    c = 2 * g + ci
                                    for j in range(sn):
                                        nc.tensor.matmul(
                                            pt[:, ci, j, :],
                                            etiles[c][:, s4 + j, :],
                                            ident[:, :],
                                            is_transpose=True,
                                            start=True,
                                            stop=True,
                                        )
                                if g == 0:
                                    nc.scalar.activation(
                                        out=et[:, 0:2, 0:sn, :],
                                        in_=pt[:, :, 0:sn, :],
                                        func=mybir.ActivationFunctionType.Copy,
                                    )
                                else:
                                    nc.scalar.activation(
                                        out=et[:, 2, 0:sn, :],
                                        in_=pt[:, 0, 0:sn, :],
                                        func=mybir.ActivationFunctionType.Copy,
                                    )
                                    nc.vector.tensor_copy(
                                        out=et[:, 3, 0:sn, :],
                                        in_=pt[:, 1, 0:sn, :],
                                    )
                            for k in range(NC):
                                nc.tensor.matmul(
                                    sc[0:NS, 0 : sn * D],
                                    wte_sel[:, t, k, :],
                                    et[:, k, 0:sn, :].rearrange(
                                        "p s d -> p (s d)"
                                    ),
                                    start=(k == 0),
                                    stop=(k == NC - 1),
                                )
                            ssb = ssp.tile([8, 4, D], f32, tag="ssb")
                            nc.scalar.activation(
                                out=ssb[0:NS, 0:sn, :],
                                in_=sc[0:NS, 0 : sn * D].rearrange(
                                    "c (s d) -> c s d", s=sn
                                ),
                                func=mybir.ActivationFunctionType.Copy,
                            )
                            sct = sctps.tile([128, 4, NS], f32, tag="sct")
                            for j in range(sn):
                                nc.tensor.matmul(
                                    sct[:, j, :],
                                    ssb[0:NS, j, :],
                                    identf[0:NS, 0:NS],
                                    is_transpose=True,
                                    start=True,
                                    stop=True,
                                )
                            nc.scalar.activation(
                                out=sc_e[:, s4 : s4 + sn, :],
                                in_=sct[:, 0:sn, :],
                                func=mybir.ActivationFunctionType.Copy,
                            )
                    # raw = sc_e[:, :, 0:4] + bias_t + w_h-score (broadcast)
                    if mode == "dma":
                        out_sb = op.tile([128, S, D], f16, tag="out_sb")
                        nc.vector.memset(out_sb[0:pmax], 0.0)
                        nc.sync.dma_start(
                            out=out.ap()[t, r0 : r0 + nr, :].rearrange(
                                "(p s) d -> p s d", p=128
                            ),
                            in_=out_sb[:, 0:S, :],
                        )
                        continue
                    raw = sp.tile([128, S, NC], f32, tag="raw")
                    nc.vector.scalar_tensor_tensor(
                        out=raw[0:pmax],
                        in0=sc_e[0:pmax, :, 0:NC],
                        scalar=bias[0:pmax, t : t + 1],
                        in1=sc_e[0:pmax, :, NC : NC + 1].broadcast_to(
                            (pmax, S, NC)
                        ),
                        op0=mybir.AluOpType.add,
                        op1=mybir.AluOpType.add,
                    )
                    # LeakyReLU(y) = max(slope*y, y), exact in one fused op
                    leaky = sp.tile([128, S, NC], f32, tag="leaky")
                    nc.vector.scalar_tensor_tensor(
                        out=leaky[0:pmax],
                        in0=raw[0:pmax],
                        scalar=NEG_SLOPE,
                        in1=raw[0:pmax],
                        op0=mybir.AluOpType.mult,
                        op1=mybir.AluOpType.max,
                    )
                    ex = sp.tile([128, S, NC], f32, tag="ex")
                    nc.scalar.activation(
                        out=ex[0:pmax],
                        in_=leaky[0:pmax],
                        func=mybir.ActivationFunctionType.Exp,
                    )
                    ssum = sp.tile([128, S, 1], f32, tag="ssum")
                    nc.vector.tensor_reduce(
                        out=ssum[0:pmax, :, 0],
                        in_=ex[0:pmax],
                        axis=mybir.AxisListType.X,
                        op=mybir.AluOpType.add,
                    )
                    rcp = sp.tile([128, S, 1], f32, tag="rcp")
                    nc.vector.reciprocal_approx_fast(rcp[0:pmax], ssum[0:pmax])

                    # --- aggregation via diagonal matmuls (fp16) -----------
                    out_sb = op.tile([128, S, D], f16, tag="out_sb")
                    if mode == "noagg":
                        nc.vector.tensor_scalar_mul(
                            out_sb[0:pmax], etiles[T][0:pmax], rcp[0:pmax, 0, 0:1]
                        )
                        nc.sync.dma_start(
                            out=out.ap()[t, r0 : r0 + nr, :].rearrange(
                                "(p s) d -> p s d", p=128
                            ),
                            in_=out_sb[:, 0:S, :],
                        )
                        continue
                    for s4 in range(0, S, agg_w):
                        sn = min(agg_w, S - s4)
                        ps = pp.tile([128, agg_w, D], f32, tag="ps")
                        for si in range(s4, s4 + sn):
                            for c in range(NC):
                                dg = dp.tile([128, 128], f16, tag="dg")
                                nc.vector.tensor_scalar(
                                    dg[0:pmax],
                                    ident[0:pmax],
                                    ex[0:pmax, si, c : c + 1],
                                    rcp[0:pmax, si, 0:1],
                                    mybir.AluOpType.mult,
                                    mybir.AluOpType.mult,
                                )
                                nc.tensor.matmul(
                                    ps[0:pmax, si - s4, :],
                                    dg[0:pmax, 0:pmax],
                                    etiles[c][0:pmax, si, :],
                                    start=(c == 0),
                                    stop=(c == NC - 1),
                                )
                        oc_i[0] += 1
                        if oc_act and oc_i[0] % oc_act == 0:
                            nc.scalar.activation(
                                out=out_sb[0:pmax, s4 : s4 + sn, :],
                                in_=ps[0:pmax, 0:sn, :],
                                func=mybir.ActivationFunctionType.Copy,
                            )
                        else:
                            nc.vector.tensor_copy(
                                out=out_sb[0:pmax, s4 : s4 + sn, :],
                                in_=ps[0:pmax, 0:sn, :],
                            )
                    # store
                    nc.sync.dma_start(
                        out=out.ap()[t, r0 : r0 + nr, :].rearrange(
                            "(p s) d -> p s d", p=128
                        ),
                        in_=out_sb[:, 0:S, :],
                    )

    nc.compile()
    return nc


def _get_nc():
    if "nc" not in _cache:
        _cache["nc"] = build_nc()
    return _cache["nc"]


def _shard(h_center, h_neigh, att_w, att_b):
    """Cast to fp16, shard the node axis over cores, pad to BSP rows."""
    hc = np.asarray(h_center, dtype=np.float32).astype(np.float16)
    hn = np.asarray(h_neigh, dtype=np.float32).astype(np.float16)
    aw = np.asarray(att_w, dtype=np.float32)
    ab = np.asarray(att_b, dtype=np.float32)
    hcp = np.zeros((NCORES, T, BSP, D), np.float16)
    hnp = np.zeros((NCORES, T, T, BSP, D), np.float16)
    for c in range(NCORES):
        sl = slice(c * BS, (c + 1) * BS)
        hcp[c, :, :BS] = hc[:, sl, :]
        hnp[c, :, :, :BS] = hn[:, :, sl, :]
    return hcp, hnp, aw, ab


def _get_runner():
    """Persistent shard_map-jitted executable over the 8 cores (compile once,
    reuse across kernel() calls; no donation so buffers are reusable)."""
    if "runner" in _cache:
        return _cache["runner"]
    import jax
    from concourse import mybir
    from concourse.bass2jax import (
        _bass_exec_p,
        install_neuronx_cc_hook,
        partition_id_tensor,
    )
    from jax.experimental.shard_map import shard_map
    from jax.sharding import Mesh, PartitionSpec, NamedSharding

    nc = _get_nc()
    install_neuronx_cc_hook()
    partition_name = nc.partition_id_tensor.name if nc.partition_id_tensor else None
    in_names, out_names, out_avals, zero_outs = [], [], [], []
    for alloc in nc.m.functions[0].allocations:
        if not isinstance(alloc, mybir.MemoryLocationSet):
            continue
        name = alloc.memorylocations[0].name
        if alloc.kind == "ExternalInput":
            if name != partition_name:
                in_names.append(name)
        elif alloc.kind == "ExternalOutput":
            shape = tuple(alloc.tensor_shape)
            dtype = mybir.dt.np(alloc.dtype)
            out_names.append(name)
            out_avals.append(jax.core.ShapedArray(shape, dtype))
            zero_outs.append(np.zeros(shape, dtype))
    all_in_names = list(in_names) + list(out_names)
    if partition_name is not None:
        all_in_names.append(partition_name)

    def _body(*args):
        operands = list(args)
        if partition_name is not None:
            operands.append(partition_id_tensor())
        return tuple(
            _bass_exec_p.bind(
                *operands,
                out_avals=tuple(out_avals),
                in_names=tuple(all_in_names),
                out_names=tuple(out_names),
                lowering_input_output_aliases=(),
                sim_require_finite=True,
                sim_require_nnan=True,
                nc=nc,
            )
        )

    devices = jax.devices()[:NCORES]
    mesh = Mesh(np.asarray(devices), ("core",))
    n_args = len(in_names) + len(out_names)
    fn = jax.jit(
        shard_map(
            _body,
            mesh=mesh,
            in_specs=(PartitionSpec("core"),) * n_args,
            out_specs=(PartitionSpec("core"),) * len(out_names),
            check_rep=False,
        ),
        keep_unused=True,
    )
    sharding = NamedSharding(mesh, PartitionSpec("core"))
    zeros_dev = [
        jax.device_put(
            np.zeros((NCORES * z.shape[0], *z.shape[1:]), z.dtype), sharding
        )
        for z in zero_outs
    ]
    _cache["runner"] = (fn, in_names, sharding, zeros_dev)
    return _cache["runner"]


def kernel(h_center, h_neigh, att_w, att_b):
    import jax

    hcp, hnp, aw, ab = _shard(h_center, h_neigh, att_w, att_b)
    try:
        fn, in_names, sharding, zeros_dev = _get_runner()
        per_name = {
            "h_center": hcp.reshape(NCORES * T, BSP, D),
            "h_neigh": hnp.reshape(NCORES * T, T, BSP, D),
            "att_w": np.broadcast_to(aw, (NCORES, T, 2 * D)).reshape(
                NCORES * T, 2 * D
            ),
            "att_b": np.broadcast_to(ab, (NCORES, T)).reshape(NCORES * T),
        }
        args = [jax.device_put(per_name[n], sharding) for n in in_names]
        out = fn(*args, *zeros_dev)
        o = np.asarray(out[0]).astype(np.float32)
        o = o.reshape(NCORES, T, BSP, D)[:, :, :BS, :]
        return np.concatenate(list(o), axis=1)
    except Exception:
        # Fallback: the stock SPMD runner (re-traces per call, always works)
        from concourse.bass_utils import run_bass_kernel_spmd

        nc = _get_nc()
        in_maps = [
            {
                "h_center": hcp[c],
                "h_neigh": hnp[c],
                "att_w": aw,
                "att_b": ab,
            }
            for c in range(NCORES)
        ]
        res = run_bass_kernel_spmd(nc, in_maps, core_ids=list(range(NCORES)))
        return np.concatenate(
            [r["out"][:, :BS].astype(np.float32) for r in res.results], axis=1
        )
